# revision 7
# baseline (speedup 1.0000x reference)
"""MoE arg-classifier (nn_ArgClassifierLayer) on 8 Trainium2 NeuronCores, v2.

Strategy (pure data-parallel over batch, no collectives):
  - Host sorts samples by expert id, deals them to 8 cores with identical
    per-expert segment layouts (capacity ceil(count/8), zero-padded), so the
    grouped GEMM becomes dense GEMMs over contiguous column ranges.
  - All activations feature-major ([feat, row]); host pre/post-transposes.
  - concat([arg, ev]) @ Wm splits into arg @ Wm_top (per entity row) +
    ev @ Wm_bot (once per sample, broadcast over 28 entities).
  - The arg-half contraction (500 features) is split by precision: 244
    features (chosen to minimize quantization error energy) run as ONE
    fp8-e4m3 DoubleRow matmul (2 k-tiles per instruction, 0.5 cyc/col);
    the other 256 run as two bf16 k-tiles.  Weights are pre-scaled by 64
    (exact in bf16; lifts fp8 weights out of subnormals); relu's positive
    homogeneity lets the 1/64 fold into W1 on the host, so on-chip math
    never sees the scale.
  - b1 is folded into the L1 matmul via a constant row planted in the
    padding of the merged activations (w1 row 500 carries b1/64).
  - Expert work in 18-sample pieces; two pieces (two experts) pair up in
    one L1 PSUM bank (PE column halves).  L2 uses a per-pair block-diagonal
    [128x32] stationary so ONE matmul computes both pieces' logits; two
    pairs (a quad = up to 4 contiguous pieces) share one L2 PSUM bank and
    drain into a [16, <=2016] staging tile -> one DMA per quad.
  - Engine balance: merge adds on DVE; merge relus cycled Act/Pool
    (GPSIMD cannot touch PSUM, so it only gets SBUF-resident relus and the
    deferred w1/w2 weight DMAs); L1 relus on Act; quad drains on Act
    (b2 rides the per-partition activation bias).
  - bf16/fp8 matmul inputs, fp32 PSUM accumulation; fp32 output.
"""

import contextlib
import math

import numpy as np
import ml_dtypes

import concourse.bass as bass
import concourse.tile as tile
from concourse import bacc, mybir
from concourse.bass_utils import run_bass_kernel_spmd

BF16 = mybir.dt.bfloat16
F8 = mybir.dt.float8e4
F32 = mybir.dt.float32
NPBF16 = ml_dtypes.bfloat16
NPF8 = ml_dtypes.float8_e4m3

B, E, D = 4096, 28, 500
M, H, R_OUT = 500, 64, 16
NEXP = 34
NCORES = 8
DP = 512  # padded merge-output dim
P = 128
KT = 4  # k-tiles of 128 over merged features (L1) / ev features (yev)
SC = 18  # samples per piece -> 504 cols <= one PSUM bank
BS = 2 * SC  # 36 samples per merge block (1008 cols)
SCALE = 64.0  # host-side weight scale (power of two: exact in bf16)

# Precision split of the 500 arg features after permutation:
#   [0, NF8)            -> fp8, one DoubleRow matmul over 2*PDR rows
#   [NF8, NF8+128*NBF)  -> bf16, NBF k-tiles
PDR = 128  # DoubleRow partition rows (contraction = 2*PDR)
NBF = 2  # bf16 k-tiles
NF8 = 500 - P * NBF  # 244 real fp8 features (pad to 2*PDR with zeros)

LAST_INFO = {}


def plan_from_evt(evt):
    evt = np.asarray(evt).astype(np.int64)
    counts = np.bincount(evt, minlength=NEXP)
    kcap = -(-counts // NCORES)
    T = int(kcap.sum())
    order = np.argsort(evt, kind="stable")
    cum = np.concatenate([[0], np.cumsum(counts)])
    assign = np.full((NCORES, T), -1, dtype=np.int64)
    pos = 0
    segs = []  # (expert, sample_start_within_core, n_samples)
    # group experts by equal segment size so consecutive pieces pair with
    # equal widths (no mc-extension waste in L1/L2)
    g_order = sorted(range(NEXP), key=lambda g: (int(kcap[g]), g))
    for g in g_order:
        kg = int(kcap[g])
        if kg == 0:
            continue
        Ig = order[cum[g] : cum[g + 1]]
        for c in range(NCORES):
            seg = Ig[c * kg : (c + 1) * kg]
            assign[c, pos : pos + len(seg)] = seg
        segs.append((g, pos, kg))
        pos += kg

    # --- static schedule: pieces -> pairs -> quads, block by block ---
    blocks = [(s, min(BS, T - s)) for s in range(0, T, BS)]
    nblocks = len(blocks)

    pairs = []  # (pieceA, pieceB | None), piece = (g, s0, ns)
    quads = []  # list of [pair_idx] (len 1 or 2)
    block_events = [[] for _ in range(nblocks + 1)]
    emitted = [False] * len(segs)
    carry = []
    pending = []

    def schedule(s_done, flush, ev):
        ready = list(carry)
        carry.clear()
        for i, (g, g0, kg) in enumerate(segs):
            if not emitted[i] and g0 + kg <= s_done:
                for p0 in range(0, kg, SC):
                    ready.append((g, g0 + p0, min(SC, kg - p0)))
                emitted[i] = True
        j = 0
        while j < len(ready):
            if j + 1 < len(ready):
                pi = len(pairs)
                pairs.append((ready[j], ready[j + 1]))
                j += 2
            elif flush:
                pi = len(pairs)
                pairs.append((ready[j], None))
                j += 1
            else:
                carry.append(ready[j])
                j += 1
                continue
            pending.append(pi)
            ev.append(("pair", pi))
            while len(pending) >= 2:
                qi = len(quads)
                quads.append([pending.pop(0), pending.pop(0)])
                ev.append(("quad", qi))

    for b in range(1, nblocks):
        schedule(blocks[b][0], False, block_events[b])
    schedule(T, True, block_events[nblocks])
    while pending:
        qi = len(quads)
        quads.append(pending[:2])
        del pending[:2]
        block_events[nblocks].append(("quad", qi))

    # map pair -> (quad index, slot within quad) for w2p/b2q layouts
    pair_slot = {}
    for qi, quad in enumerate(quads):
        for j, pi in enumerate(quad):
            pair_slot[pi] = (qi, j)

    return dict(
        kcap=kcap, T=T, R=T * E, assign=assign, segs=segs,
        pairs=pairs, quads=quads, block_events=block_events, nblocks=nblocks,
        pair_slot=pair_slot, blocks=blocks, g_order=g_order,
    )


def build_nc(plan, loop_n=None, hint=True, repeat=1):
    T, R = plan["T"], plan["R"]
    pairs, quads = plan["pairs"], plan["quads"]
    pair_slot = plan["pair_slot"]
    block_events, nblocks = plan["block_events"], plan["nblocks"]
    blocks = plan["blocks"]
    # w1 columns are stored in SEGMENT order so the early-fetched half is
    # exactly what the earliest pairs need
    slot = {g: s for s, g in enumerate(plan["g_order"])}
    NQ = len(quads)
    NPAIR = len(pairs)
    BC = BS * E  # 1008 cols per merge block

    nc = bacc.Bacc("TRN2", target_bir_lowering=False, debug=False,
                   num_devices=NCORES)
    argt8 = nc.dram_tensor("argt8", [P, 2 * R], F8, kind="ExternalInput").ap()
    argt16 = nc.dram_tensor("argt16", [P, NBF * R], BF16, kind="ExternalInput").ap()
    wma8 = nc.dram_tensor("wma8", [P, 2 * DP], F8, kind="ExternalInput").ap()
    wma16 = nc.dram_tensor("wma16", [P, NBF * DP], BF16, kind="ExternalInput").ap()
    # wpre = [wme k0 | evtt k0 | bm] + [wme k1..3 | evtt k1..3] in one tensor,
    # fetched as two DMAs so yev's first sweep can start early.
    W0C = DP + T + KT  # cols in the first chunk
    W1C = 3 * (DP + T)
    wpre = nc.dram_tensor("wpre", [P, W0C + W1C], BF16, kind="ExternalInput").ap()
    w1t = nc.dram_tensor("w1t", [P, KT * NEXP * H], BF16, kind="ExternalInput").ap()
    w2p = nc.dram_tensor("w2p", [P, 64 * max(NPAIR, 1)], BF16, kind="ExternalInput").ap()
    b2q = nc.dram_tensor("b2q", [P, max(NQ, 1)], F32, kind="ExternalInput").ap()
    out = nc.dram_tensor("out", [R_OUT, R], F32, kind="ExternalOutput").ap()

    relu_cycle = ("act", "pool", "act", "pool")

    with tile.TileContext(nc) as tc:
        with (
            tc.tile_pool(name="wpool", bufs=1) as wpool,
            tc.tile_pool(name="big", bufs=1) as big,
            tc.tile_pool(name="io", bufs=5) as io,
            tc.tile_pool(name="et", bufs=3) as etpool,
            tc.tile_pool(name="eto", bufs=4) as etopool,
            tc.tile_pool(name="psm", bufs=2, space="PSUM") as psm,
            tc.tile_pool(name="pse", bufs=4, space="PSUM") as pse,
        ):
            # --- static SBUF tiles -------------------------------------
            wpre0 = wpool.tile([P, W0C], BF16, tag="wpre0")
            wpre1 = wpool.tile([P, W1C], BF16, tag="wpre1")
            wma8_sb = wpool.tile([P, 2 * DP], F8, tag="wma8")
            wma16_sb = wpool.tile([P, NBF * DP], BF16, tag="wma16")
            w1_sb = wpool.tile([P, KT * NEXP * H], BF16, tag="w1")
            w2p_sb = wpool.tile([P, 64 * max(NPAIR, 1)], BF16, tag="w2p")
            b2q_sb = wpool.tile([P, max(NQ, 1)], F32, tag="b2q")
            yev_sb = [wpool.tile([P, T], BF16, tag=f"yev{m}", name=f"yev{m}")
                      for m in range(KT)]
            # +504 zero-padded cols so unequal pairs can extend reads past R
            RP = R + 504
            merged = [
                big.tile([P, RP], BF16, tag=f"merged{m}", name=f"merged{m}")
                for m in range(KT)
            ]
            for m in range(KT):
                nc.vector.memset(merged[m][:, R:RP], 0.0)

            # wpre0 layout: [wme-k0-m0 | evt-k0 | wme-k0-m123 | bm] so the
            # first yev matmul's operands are a contiguous prefix (small
            # first DMA -> PE starts ~0.3us sooner)
            def wme_v(k, m):  # [P, 128] stationary slice for m-tile m
                if k == 0:
                    if m == 0:
                        return wpre0[:, 0:P]
                    c0 = P + T + (m - 1) * P
                    return wpre0[:, c0 : c0 + P]
                c0 = (k - 1) * (DP + T)
                return wpre1[:, c0 + m * P : c0 + (m + 1) * P]

            def evt_v(k, q0, qn):  # [P, qn] moving slice
                if k == 0:
                    return wpre0[:, P + q0 : P + q0 + qn]
                c0 = (k - 1) * (DP + T) + DP
                return wpre1[:, c0 + q0 : c0 + q0 + qn]

            bm_col = lambda m: wpre0[:, DP + T + m : DP + T + m + 1]

            PF = 2  # arg blocks prefetched in the preamble
            at8_tiles = {}
            at16_tiles = {}
            # w1 split: first NEA experts (A, needed by early pairs) fetched in
            # the preamble; the rest (B) interleaved into the block loop.
            NEA = NEXP // 2
            CA = NEA * H  # A-chunk cols per k-tile
            CB = NEXP * H - CA

            def w1_dma(k, half):
                c0 = k * NEXP * H + (0 if half == 0 else CA)
                cw = CA if half == 0 else CB
                nc.sync.dma_start(w1_sb[:, c0 : c0 + cw], w1t[:, c0 : c0 + cw])

            def issue_block_dma(b, eng=None):
                eng = eng or nc.sync
                s0, ns_blk = blocks[b]
                col0 = s0 * E
                bcols = ns_blk * E
                a8 = io.tile([P, 2 * BC], F8, tag="arg8")
                a16 = io.tile([P, NBF * BC], BF16, tag="arg16")
                v8 = a8[:, :]
                dst8 = bass.AP(v8.tensor, v8.offset, [v8.ap[0], [BC, 2], [1, bcols]])
                s8 = argt8[:, :]
                src8 = bass.AP(s8.tensor, s8.offset + col0,
                               [s8.ap[0], [R, 2], [1, bcols]])
                eng.dma_start(dst8, src8)
                v16 = a16[:, :]
                dst16 = bass.AP(v16.tensor, v16.offset,
                                [v16.ap[0], [BC, NBF], [1, bcols]])
                s16 = argt16[:, :]
                src16 = bass.AP(s16.tensor, s16.offset + col0,
                                [s16.ap[0], [R, NBF], [1, bcols]])
                eng.dma_start(dst16, src16)
                at8_tiles[b] = a8
                at16_tiles[b] = a16

            # hand-ordered single-queue (SP) preamble: FIFO on DMA_ENGINES
            # then matches program order exactly.
            nc.sync.dma_start(wpre0[:], wpre[:, :W0C])
            CK = DP + T  # one (wme k, evt k) chunk
            nc.sync.dma_start(wpre1[:, :CK], wpre[:, W0C : W0C + CK])
            nc.sync.dma_start(wpre1[:, CK:], wpre[:, W0C + CK :])
            nc.sync.dma_start(wma8_sb[:], wma8[:, :])
            nc.sync.dma_start(wma16_sb[:], wma16[:, :])
            issue_block_dma(0)
            if nblocks > 1:
                issue_block_dma(1)
            w1_dma(0, 0)
            w1_dma(1, 0)
            w1_dma(2, 0)
            w1_dma(3, 0)
            nc.sync.dma_start(w2p_sb[:], w2p[:, :])
            nc.sync.dma_start(b2q_sb[:], b2q[:, :])

            # PE p-state warmup: a few throwaway matmuls over the zeroed
            # pad columns while the first weight DMAs are in flight, so real
            # work starts at full clock.  Results land in a scratch pse tile
            # that is recycled before any real psum use.
            warm = pse.tile([P, 512], F32, tag="pse", name="warm")
            wsrc = merged[0][:, R : R + 504]
            for wn in (504, 504, 504, 252):
                nc.tensor.matmul(
                    warm[:, :wn],
                    merged[0][:, R : R + P],
                    wsrc[:, :wn],
                    start=True, stop=True, skip_group_check=True,
                )

            pair_live = {}  # pair_idx -> (pieceA, pieceB, ht)
            drain_i = [0]

            tail_pair_i = [0]

            def emit_pair(pi, tail=False):
                pa, pb = pairs[pi]
                ga, a0, na = pa
                ca = na * E
                mc = ca
                if pb is not None:
                    gb, b0, nb = pb
                    cb = nb * E
                    mc = max(ca, cb)
                hp = pse.tile([P, 512], F32, tag="pse", name="hp")
                for k in range(KT):
                    nc.tensor.matmul(
                        hp[0:H, :mc],
                        w1_sb[:, k * NEXP * H + slot[ga] * H : k * NEXP * H + (slot[ga] + 1) * H],
                        merged[k][:, a0 * E : a0 * E + mc],
                        start=(k == 0), stop=(k == KT - 1),
                        tile_position=(0, 0), skip_group_check=True,
                    )
                    if pb is not None:
                        nc.tensor.matmul(
                            hp[H : 2 * H, :mc],
                            w1_sb[:, k * NEXP * H + slot[gb] * H : k * NEXP * H + (slot[gb] + 1) * H],
                            merged[k][:, b0 * E : b0 * E + mc],
                            start=(k == 0), stop=(k == KT - 1),
                            tile_position=(0, H), skip_group_check=True,
                        )
                ht = etpool.tile([P, 512], BF16, tag="htile")
                rows = P if pb is not None else H
                if tail and tail_pair_i[0] % 2 == 0:
                    tail_pair_i[0] += 1
                    nc.vector.tensor_scalar_max(ht[:rows, :mc],
                                                hp[:rows, :mc], 0.0)
                else:
                    if tail:
                        tail_pair_i[0] += 1
                    nc.scalar.activation(ht[:rows, :mc], hp[:rows, :mc],
                                         mybir.ActivationFunctionType.Relu)
                pair_live[pi] = (pa, pb, ht)

            def emit_quad(qi, tail=False):
                quad = quads[qi]
                op = pse.tile([P, 512], F32, tag="pse", name="op")
                gcol0 = None
                outs = []  # (psum_row0, global_col0, cols)
                for j, pi in enumerate(quad):
                    pa, pb, ht = pair_live.pop(pi)
                    ga, a0, na = pa
                    ca = na * E
                    mc = ca
                    cb = 0
                    if pb is not None:
                        gb, b0, nb = pb
                        cb = nb * E
                        mc = max(ca, cb)
                        # block-diagonal [128x64] stationary (A cols 0-15,
                        # B cols 32-47; zero cols elsewhere): both pieces in
                        # ONE matmul, psum bases stay 32-aligned
                        nc.tensor.matmul(
                            op[64 * j : 64 * j + 64, :mc],
                            w2p_sb[:, pi * 64 : (pi + 1) * 64],
                            ht[:, :mc],
                            start=True, stop=True,
                            tile_position=(0, 64 * j), skip_group_check=True,
                        )
                    else:
                        # lone piece: 64-row stationary so ht's unwritten
                        # B-half never enters the PE
                        nc.tensor.matmul(
                            op[64 * j : 64 * j + R_OUT, :mc],
                            w2p_sb[0:H, pi * 64 : pi * 64 + R_OUT],
                            ht[0:H, :mc],
                            start=True, stop=True,
                            tile_position=(0, 64 * j), skip_group_check=True,
                        )
                    if gcol0 is None:
                        gcol0 = a0 * E
                    outs.append((64 * j, a0 * E, ca))
                    if pb is not None:
                        outs.append((64 * j + 32, b0 * E, cb))
                # Steady state: one staging tile + one DMA per PAIR.
                # Tail: one tile + one DMA per PIECE, engines alternating, so
                # nothing serializes on a shared dst tile at the very end.
                np_out = 0
                for j, pi in enumerate(quad):
                    pouts = outs[np_out : np_out + (2 if pairs[pi][1] is not None else 1)]
                    np_out += len(pouts)
                    groups = ([[p] for p in pouts] if tail else [pouts])
                    for gi, grp in enumerate(groups):
                        oq = etopool.tile([R_OUT, 2 * 504], F32, tag="oq",
                                          name="oq")
                        gc0 = grp[0][1]
                        span = 0
                        for di, (r0, c0, cc) in enumerate(grp):
                            lc = c0 - gc0
                            use_dve = tail and (gi % 2 == 1)
                            if not use_dve:
                                nc.scalar.activation(
                                    oq[:, lc : lc + cc],
                                    op[r0 : r0 + R_OUT, :cc],
                                    mybir.ActivationFunctionType.Identity,
                                    bias=b2q_sb[r0 : r0 + R_OUT, qi : qi + 1],
                                )
                            else:
                                bcol = b2q_sb[r0 : r0 + R_OUT, qi : qi + 1]
                                bb = bass.AP(bcol.tensor, bcol.offset,
                                             [bcol.ap[0], [0, cc]])
                                nc.vector.tensor_tensor(
                                    oq[:, lc : lc + cc],
                                    op[r0 : r0 + R_OUT, :cc],
                                    bb, mybir.AluOpType.add,
                                )
                            span = max(span, lc + cc)
                        nc.sync.dma_start(
                            out[:, gc0 : gc0 + span], oq[:, :span])

            def body_ctx():
                if loop_n is not None:
                    hints = (
                        (mybir.EngineType.PE, mybir.EngineType.Activation,
                         mybir.EngineType.DVE, mybir.EngineType.SP,
                         mybir.EngineType.Pool)
                        if hint else ()
                    )
                    return tc.For_i(0, loop_n, 1, hint_engines=hints)
                return contextlib.nullcontext(0)

            relu_i = [0]

            def emit_body():
                # --- yev = Wm_bot.T @ ev.T + bm.  PSUM comes from the pse
                # pool (pairs/quads don't need it yet), so the psm pool stays
                # free and block-0's merge can start the moment its data
                # lands, overlapping yev's Act drains. -------------------
                qs_t = [(q0, min(504, T - q0)) for q0 in range(0, T, 504)]
                for m in range(KT):
                    pst = [pse.tile([P, 512], F32, tag="pse", name=f"yv{m}q{qi}")
                           for qi in range(len(qs_t))]
                    for k in range(KT):
                        for qi, (q0, qn) in enumerate(qs_t):
                            nc.tensor.matmul(
                                pst[qi][:, :qn],
                                wme_v(k, m),
                                evt_v(k, q0, qn),
                                start=(k == 0), stop=(k == KT - 1),
                            )
                    for qi, (q0, qn) in enumerate(qs_t):
                        nc.scalar.activation(
                            yev_sb[m][:, q0 : q0 + qn],
                            pst[qi][:, :qn],
                            mybir.ActivationFunctionType.Identity,
                            bias=bm_col(m),
                        )

                # --- merge blocks + expert work --------------------------
                for b in range(nblocks):
                    if b not in at8_tiles:
                        issue_block_dma(b)
                        if 0 <= b - PF < KT:
                            w1_dma(b - PF, 1)
                    a8, a16 = at8_tiles.pop(b), at16_tiles.pop(b)
                    s0, ns_blk = blocks[b]
                    col0 = s0 * E
                    bcols = ns_blk * E
                    qs = [(q0, min(504, bcols - q0)) for q0 in range(0, bcols, 504)]
                    for m in range(KT):
                        pt = psm.tile([P, 1024], F32, tag="psm", name="pt")
                        for qi, (q0, qn) in enumerate(qs):
                            v8 = a8[:, :]
                            rhs8 = bass.AP(v8.tensor, v8.offset + q0,
                                           [v8.ap[0], [BC, 2], [1, qn]])
                            w8 = wma8_sb[:, :]
                            lhs8 = bass.AP(w8.tensor, w8.offset + m * P,
                                           [w8.ap[0], [DP, 2], [1, P]])
                            nc.tensor.matmul(
                                pt[:, qi * 512 : qi * 512 + qn],
                                lhs8, rhs8,
                                start=True, stop=False,
                                perf_mode=mybir.MatmulPerfMode.DoubleRow,
                                skip_group_check=True,
                            )
                            for kb in range(NBF):
                                nc.tensor.matmul(
                                    pt[:, qi * 512 : qi * 512 + qn],
                                    wma16_sb[:, kb * DP + m * P : kb * DP + (m + 1) * P],
                                    a16[:, kb * BC + q0 : kb * BC + q0 + qn],
                                    start=False, stop=(kb == NBF - 1),
                                    skip_group_check=True,
                                )
                        dst = merged[m][:, col0 : col0 + bcols]
                        if b == nblocks - 1 and len(qs) > 1:
                            # last block: per-chunk epilogue so tail pairs
                            # whose pieces end in the first chunk can start
                            # their L1 without waiting the full-width ops
                            for qi, (q0, qn) in enumerate(qs):
                                qsmp = qn // E
                                dd = merged[m][:, col0 + q0 : col0 + q0 + qn]
                                pv = pt[:, qi * 512 : qi * 512 + qn]
                                ps3 = pv.rearrange("p (s e) -> p s e", e=E)
                                d3 = dd.rearrange("p (s e) -> p s e", e=E)
                                yv = yev_sb[m][:, s0 + qi * SC : s0 + qi * SC + qsmp]
                                y3 = bass.AP(yv.tensor, yv.offset,
                                             list(yv.ap) + [[0, E]])
                                nc.vector.tensor_tensor(d3, ps3, y3,
                                                        mybir.AluOpType.add)
                                if qi % 2 == 0:
                                    nc.scalar.activation(
                                        dd, dd,
                                        mybir.ActivationFunctionType.Relu)
                                else:
                                    nc.gpsimd.tensor_scalar_max(dd, dd, 0.0)
                            relu_i[0] += 1
                            continue
                        if ns_blk == BS:
                            # one fused add over both psum banks (4D AP)
                            pv = pt[:, :504]
                            p4 = bass.AP(pv.tensor, pv.offset,
                                         [pv.ap[0], [512, 2], [E, SC], [1, E]])
                            d4 = bass.AP(dst.tensor, dst.offset,
                                         [dst.ap[0], [504, 2], [E, SC], [1, E]])
                            yv = yev_sb[m][:, s0 : s0 + BS]
                            y4 = bass.AP(yv.tensor, yv.offset,
                                         [yv.ap[0], [SC, 2], [1, SC], [0, E]])
                            nc.vector.tensor_tensor(d4, p4, y4, mybir.AluOpType.add)
                        else:
                            for qi, (q0, qn) in enumerate(qs):
                                qsmp = qn // E
                                dd = merged[m][:, col0 + q0 : col0 + q0 + qn]
                                pv = pt[:, qi * 512 : qi * 512 + qn]
                                ps3 = pv.rearrange("p (s e) -> p s e", e=E)
                                d3 = dd.rearrange("p (s e) -> p s e", e=E)
                                yv = yev_sb[m][:, s0 + qi * SC : s0 + qi * SC + qsmp]
                                y3 = bass.AP(yv.tensor, yv.offset,
                                             list(yv.ap) + [[0, E]])
                                nc.vector.tensor_tensor(d3, ps3, y3,
                                                        mybir.AluOpType.add)
                        eng = relu_cycle[relu_i[0] % len(relu_cycle)]
                        relu_i[0] += 1
                        if eng == "act":
                            nc.scalar.activation(dst, dst,
                                                 mybir.ActivationFunctionType.Relu)
                        else:
                            nc.gpsimd.tensor_scalar_max(dst, dst, 0.0)
                    if b >= 1:
                        for kind, idx in block_events[b]:
                            (emit_pair if kind == "pair" else emit_quad)(idx)
                for kind, idx in block_events[nblocks]:
                    if kind == "pair":
                        emit_pair(idx, tail=True)
                    else:
                        emit_quad(idx, tail=True)

            with body_ctx():
                for _rep in range(repeat):
                    emit_body()

    nc.compile()
    return nc


def select_fp8_features(Wm):
    """Pick NF8 arg features for fp8 minimizing quantization error energy."""
    Wt = (SCALE * Wm[:D]).astype(np.float32)  # [D, M]
    W8 = Wt.astype(NPF8).astype(np.float32)
    werr = ((Wt - W8) ** 2).sum(axis=1)  # weight-quant energy per feature
    # activation-quant energy per feature: E[(a - fp8(a))^2] * sum_m w8^2.
    # a ~ N(0,1) iid, so E[da^2] is a constant; estimate it once.
    a = np.linspace(-4, 4, 4097, dtype=np.float32)
    w = np.exp(-0.5 * a * a)
    da2 = ((a - a.astype(NPF8).astype(np.float32)) ** 2 * w).sum() / w.sum()
    aerr = da2 * (W8**2).sum(axis=1) / (SCALE**2) * 1.0
    energy = werr / (SCALE**2) + aerr
    order = np.argsort(energy)
    f8 = np.sort(order[:NF8])
    f16 = np.sort(order[NF8:])
    return np.concatenate([f8, f16])


def make_in_maps(inputs, plan):
    arg = np.asarray(inputs["arg_mention_embeds"], dtype=np.float32)
    ev = np.asarray(inputs["event_mention_embed"], dtype=np.float32)
    Wm = np.asarray(inputs["Wm"], dtype=np.float32)
    bm = np.asarray(inputs["bm"], dtype=np.float32)
    W1 = np.asarray(inputs["W1"], dtype=np.float32)
    b1 = np.asarray(inputs["b1"], dtype=np.float32)
    W2 = np.asarray(inputs["W2"], dtype=np.float32)
    b2 = np.asarray(inputs["b2"], dtype=np.float32)
    T, R = plan["T"], plan["R"]
    assign = plan["assign"]
    quads, pairs = plan["quads"], plan["pairs"]
    NQ, NPAIR = len(quads), len(pairs)

    perm = select_fp8_features(Wm)

    # --- weights (shared by all cores) --------------------------------
    Wma = (SCALE * Wm[:D][perm]).astype(np.float32)  # [500, M] permuted
    wma8_np = np.zeros((P, 2 * DP), NPF8)
    for t in range(2):
        rows = Wma[t * P : min((t + 1) * P, NF8)]
        wma8_np[: len(rows), t * DP : t * DP + M] = rows[:, :M].astype(NPF8)
    wma16_np = np.zeros((P, NBF * DP), NPBF16)
    for t in range(NBF):
        rows = Wma[NF8 + t * P : NF8 + (t + 1) * P]
        wma16_np[: len(rows), t * DP : t * DP + M] = rows[:, :M].astype(NPBF16)

    Wme = (SCALE * Wm[D:]).astype(NPBF16)  # [500, M]
    bm_pad = np.zeros(DP, np.float32)
    bm_pad[:M] = SCALE * bm
    bm_pad[M] = SCALE  # ones-row carries b1 via w1 row 500
    W0C = DP + T + KT
    W1C = 3 * (DP + T)
    wpre_np = np.zeros((P, W0C + W1C), NPBF16)

    def wme_kt(k):
        wk = np.zeros((P, DP), NPBF16)
        rows = Wme[k * P : min((k + 1) * P, D)]
        wk[: len(rows), :M] = rows[:, :M]
        return wk

    # evt per-core below; weights common here.
    # wpre0 region: [wme-k0-m0 | evt-k0 (per-core) | wme-k0-m123 | bm]
    wk0 = wme_kt(0)
    wpre_np[:, :P] = wk0[:, :P]
    wpre_np[:, P + T : DP + T] = wk0[:, P:]
    wpre_np[:, DP + T : W0C] = np.ascontiguousarray(
        bm_pad.reshape(KT, P).T.astype(NPBF16))
    for k in range(1, KT):
        c0 = W0C + (k - 1) * (DP + T)
        wpre_np[:, c0 : c0 + DP] = wme_kt(k)

    g_order = plan["g_order"]
    W1p = W1[g_order]  # segment-ordered experts
    b1p = b1[g_order]
    w1t_np = np.zeros((P, KT * NEXP * H), NPBF16)
    w1f = np.zeros((DP, NEXP * H), np.float32)
    w1f[:M] = W1p.transpose(1, 0, 2).reshape(M, NEXP * H) / SCALE
    w1f[M] = b1p.reshape(NEXP * H) / SCALE
    for k in range(KT):
        w1t_np[:, k * NEXP * H : (k + 1) * NEXP * H] = (
            w1f[k * P : (k + 1) * P].astype(NPBF16))

    w2p_np = np.zeros((P, 64 * max(NPAIR, 1)), NPBF16)
    b2q_np = np.zeros((P, max(NQ, 1)), np.float32)
    for pi, (pa, pb) in enumerate(pairs):
        w2p_np[:H, pi * 64 : pi * 64 + R_OUT] = W2[pa[0]].astype(NPBF16)
        if pb is not None:
            w2p_np[H:, pi * 64 + 32 : pi * 64 + 48] = W2[pb[0]].astype(NPBF16)
    for qi, quad in enumerate(quads):
        for j, pi in enumerate(quad):
            pa, pb = pairs[pi]
            b2q_np[64 * j : 64 * j + R_OUT, qi] = b2[pa[0]]
            if pb is not None:
                b2q_np[64 * j + 32 : 64 * j + 32 + R_OUT, qi] = b2[pb[0]]

    # --- per-core activations ----------------------------------------
    argp = arg[:, :, perm]  # permuted features
    in_maps = []
    for c in range(NCORES):
        idx = assign[c]
        mask = idx >= 0
        ac = np.zeros((T, E, D), np.float32)
        ac[mask] = argp[idx[mask]]
        acf = ac.reshape(T * E, D).T  # [D(feat), R]
        argt8_np = np.zeros((P, 2 * R), NPF8)
        for t in range(2):
            rows = acf[t * P : min((t + 1) * P, NF8)]
            argt8_np[: len(rows), t * R : t * R + R] = rows.astype(NPF8)
        argt16_np = np.zeros((P, NBF * R), NPBF16)
        for t in range(NBF):
            rows = acf[NF8 + t * P : NF8 + (t + 1) * P]
            argt16_np[: len(rows), t * R : t * R + R] = rows.astype(NPBF16)

        evc = np.zeros((T, D), np.float32)
        evc[mask] = ev[idx[mask], 0]
        evf = evc.T  # [D, T]
        wp = wpre_np.copy()
        wp[:, P : P + T] = evf[0:P].astype(NPBF16)
        for k in range(1, KT):
            c0 = W0C + (k - 1) * (DP + T) + DP
            rows = evf[k * P : min((k + 1) * P, D)]
            wp[: len(rows), c0 : c0 + T] = rows.astype(NPBF16)
        in_maps.append(
            dict(
                argt8=argt8_np, argt16=argt16_np, wma8=wma8_np,
                wma16=wma16_np, wpre=wp, w1t=w1t_np, w2p=w2p_np, b2q=b2q_np,
            )
        )
    return in_maps


def assemble_output(results, plan):
    T = plan["T"]
    assign = plan["assign"]
    res = np.zeros((B, E, R_OUT), np.float32)
    for c in range(NCORES):
        oc = np.asarray(results[c]["out"])  # [16, R]
        oc = oc.reshape(R_OUT, T, E).transpose(1, 2, 0)
        idx = assign[c]
        mask = idx >= 0
        res[idx[mask]] = oc[mask]
    return res


def kernel(**inputs) -> np.ndarray:
    plan = plan_from_evt(inputs["evt_type_list"])
    nc = build_nc(plan)
    in_maps = make_in_maps(inputs, plan)
    res = run_bass_kernel_spmd(nc, in_maps, core_ids=list(range(NCORES)))
    LAST_INFO["plan"] = plan
    LAST_INFO["exec_time_ns"] = res.exec_time_ns
    return assemble_output(res.results, plan)
